# revision 4
# baseline (speedup 1.0000x reference)
"""LocalAttention Bass kernel for Trainium2 (8 NeuronCores) — fp16, software-pipelined (final).

v4 = v3 (fp16 everywhere in SBUF/HBM, fp32 PSUM) + software-pipelined PE
stream + mask on DVE only.

Why: the PE engine queue is in-order. In v3 the per-group program order
was S(g), PV(g), S(g+1), ... so PV(g) — which must wait for exp(g) on
ACT and mask(g) on DVE/GPSIMD — blocked S(g+1) behind it and the PE sat
idle for the whole exp+mask latency every group (~2.5us/group measured,
~160us total). v4 issues S(g+1) BEFORE PV(g) so the PE streams Q@K of
the next group while the softmax of the current one is still on the
other engines. The GPSIMD mask (measured ~2us/group, it is a slow
engine for this) moves to DVE, whose fp16 2x/4x modes make it ~0.2us.

Device algorithm per head and 128-token window w (see v1 docstring):
  S^T[k,q] = K_w' @ [Q_w | Q_{w+1}]   (PSUM pairblock [T1(w) | T0(w+1)])
  P = exp(S^T / sqrt(D)) * causal01   (ACT exp, DVE mask on T1 halves)
  [O^T; r] = [V|1]^T @ P              (2 matmuls/window, ones col = rowsum)
Host divides O^T by r and transposes back.
"""

import numpy as np

F16 = np.float16

B, H, T, D = 4, 8, 8192, 64
W = 128                     # window size
WIN = T // W                # 64 windows per head
NCORES = 8
BH = B * H                  # 32
BH_PER_CORE = BH // NCORES  # 4
NPAIR = BH_PER_CORE // 2    # 2 head pairs per core
CHUNK_W = 32                # windows per load chunk
NCHUNK = WIN // CHUNK_W     # 2
G = 4                       # windows per softmax group (PSUM tile = [128, 1024])
SCALE = float(D) ** -0.5

QBUFS = 3
KBUFS = 3
VBUFS = 6
OBUFS = 3
PBUFS = 6
OPSUM_BUFS = 4
DEPTH = 2   # software-pipeline depth (S of group j+DEPTH issues before PV of j)

_nc_cache = {}
last_perf = None


def _build_nc(skip=(), reps=1):
    import concourse.tile as tile
    from concourse import bacc
    from concourse import mybir
    from contextlib import ExitStack

    f32 = mybir.dt.float32
    f16 = mybir.dt.float16
    Exp = mybir.ActivationFunctionType.Exp
    mult = mybir.AluOpType.mult

    nc = bacc.Bacc()
    qTp = nc.dram_tensor("qTp", [NPAIR, W, (WIN + 1) * W], f16,
                         kind="ExternalInput")
    kT = nc.dram_tensor("kT", [NPAIR, W, T], f16, kind="ExternalInput")
    vp = nc.dram_tensor("vp", [BH_PER_CORE, W, (WIN + 1) * (D + 1)], f16,
                        kind="ExternalInput")
    mask = nc.dram_tensor("mask01", [W, W], f16, kind="ExternalInput")
    outT = nc.dram_tensor("outT", [BH_PER_CORE, D + 1, T], f16,
                          kind="ExternalOutput")

    with tile.TileContext(nc) as tc, ExitStack() as ctx:
        cpool = ctx.enter_context(tc.tile_pool(name="cpool", bufs=1))
        qpool = ctx.enter_context(tc.tile_pool(name="qpool", bufs=QBUFS))
        kpool = ctx.enter_context(tc.tile_pool(name="kpool", bufs=KBUFS))
        vpool = ctx.enter_context(tc.tile_pool(name="vpool", bufs=VBUFS))
        opool = ctx.enter_context(tc.tile_pool(name="opool", bufs=OBUFS))
        ppool = ctx.enter_context(tc.tile_pool(name="ppool", bufs=PBUFS))
        spsum = ctx.enter_context(tc.tile_pool(name="spsum", bufs=2,
                                               space="PSUM"))
        opsum = ctx.enter_context(tc.tile_pool(name="opsum", bufs=OPSUM_BUFS,
                                               space="PSUM"))

        mtile = cpool.tile([W, W], f16)
        nc.sync.dma_start(mtile[:], mask[:])
        z128 = cpool.tile([W, W], f16)       # P for the all-masked pad window
        nc.vector.memset(z128[:], 0.0)

        mm = nc.tensor.matmul

        def emit_front(st, qc, kc, w0, hb):
            """S matmuls + exp + mask for one (group, head) stage."""
            sp = spsum.tile([W, G * 2 * W], f32, tag="sp")
            if "smm" not in skip:
                for i in range(G):
                    wl = w0 + i
                    mm(sp[:, i * 256:(i + 1) * 256],
                       kc[hb:hb + 64, wl * W:(wl + 1) * W],
                       qc[hb:hb + 64, wl * W:(wl + 2) * W],
                       start=True, stop=True)
            pt = ppool.tile([W, G * 2 * W], f16, tag="pt")
            if "exp" not in skip:
                nc.scalar.activation(pt[:], sp[:], Exp, scale=SCALE)
            # causal mask on T1 blocks (cols 0, 256, 512, 768)
            pt3 = pt[:].rearrange("p (g x) -> p g x", x=2 * W)
            t1 = pt3[:, :, 0:W]
            mb = mtile[:, None, :].to_broadcast([W, G, W])
            if "mask" not in skip:
                nc.vector.tensor_tensor(t1, t1, mb, mult)
            st["pt"] = pt

        def emit_back(st, pt_prev, vc, oc, w0, h):
            """PV matmuls + PSUM->SBUF copy for one stage."""
            pt = st["pt"]
            op = opsum.tile([D + 1, G * W], f32, tag="op")
            if "pv" not in skip:
                for i in range(G):
                    wl = w0 + i
                    if i > 0:
                        t0src = pt[:, i * 256 - W:i * 256]
                    elif pt_prev[h] is not None:
                        t0src = pt_prev[h][:, G * 256 - W:G * 256]
                    else:
                        t0src = z128[:]
                    mm(op[:, i * W:(i + 1) * W],
                       vc[:, wl * (D + 1):(wl + 1) * (D + 1)],
                       t0src, start=True, stop=False)
                    mm(op[:, i * W:(i + 1) * W],
                       vc[:, (wl + 1) * (D + 1):(wl + 2) * (D + 1)],
                       pt[:, i * 256:i * 256 + W],
                       start=False, stop=True)
            if "ocopy" not in skip:
                nc.vector.tensor_copy(oc[:, w0 * W:(w0 + G) * W], op[:])
            pt_prev[h] = pt

        def body():
            for p in range(NPAIR):
                pt_prev = [None, None]
                for c in range(NCHUNK):
                    c0 = c * CHUNK_W * W
                    qc = qpool.tile([W, (CHUNK_W + 1) * W], f16, tag="qc")
                    kc = kpool.tile([W, CHUNK_W * W], f16, tag="kc")
                    if "loads" not in skip:
                        nc.sync.dma_start(
                            qc[:], qTp[p, :, c0:c0 + (CHUNK_W + 1) * W])
                        nc.sync.dma_start(kc[:], kT[p, :, c0:c0 + CHUNK_W * W])
                    vcs = []
                    ocs = []
                    for h in range(2):
                        vc = vpool.tile([W, (CHUNK_W + 1) * (D + 1)], f16,
                                        tag="vc")
                        if "loads" not in skip:
                            v0 = c * CHUNK_W * (D + 1)
                            nc.sync.dma_start(
                                vc[:],
                                vp[2 * p + h, :,
                                   v0:v0 + (CHUNK_W + 1) * (D + 1)])
                        vcs.append(vc)
                        oc = opool.tile([D + 1, CHUNK_W * W], f16, tag="oc")
                        ocs.append(oc)

                    # software pipeline: front(j+1) issues before back(j) so
                    # the PE streams next group's S while this group's
                    # exp/mask are still on ACT/DVE.
                    stages = [(g, h) for g in range(CHUNK_W // G)
                              for h in range(2)]
                    st = [{} for _ in stages]
                    d = DEPTH
                    for idx in range(len(stages) + d):
                        if idx < len(stages):
                            g, h = stages[idx]
                            emit_front(st[idx], qc, kc, g * G, h * 64)
                        if idx >= d:
                            g, h = stages[idx - d]
                            emit_back(st[idx - d], pt_prev, vcs[h], ocs[h],
                                      g * G, h)

                    if "store" not in skip:
                        for h in range(2):
                            nc.sync.dma_start(
                                outT[2 * p + h, :, c0:c0 + CHUNK_W * W],
                                ocs[h][:])

        if reps > 1:
            engs = (mybir.EngineType.PE, mybir.EngineType.Activation,
                    mybir.EngineType.DVE, mybir.EngineType.SP,
                    mybir.EngineType.Pool)
            with tc.For_i(0, reps, 1, hint_engines=engs):
                body()
        else:
            body()
    nc.finalize()
    return nc


def _prep_core_inputs(q2, k2, v2, core):
    s0 = core * BH_PER_CORE
    qTp = np.zeros((NPAIR, W, (WIN + 1) * W), F16)
    kTp = np.zeros((NPAIR, W, T), F16)
    for p in range(NPAIR):
        for h in range(2):
            bh = s0 + 2 * p + h
            qTp[p, h * 64:(h + 1) * 64, :T] = q2[bh].T.astype(F16)
            kTp[p, h * 64:(h + 1) * 64, :] = k2[bh].T.astype(F16)
    vr = v2[s0:s0 + BH_PER_CORE].reshape(
        BH_PER_CORE, WIN, W, D).transpose(0, 2, 1, 3)
    vp = np.zeros((BH_PER_CORE, W, WIN + 1, D + 1), F16)
    vp[:, :, 1:, :D] = vr.astype(F16)
    vp[:, :, :, D] = 1.0
    vp = np.ascontiguousarray(vp.reshape(BH_PER_CORE, W, (WIN + 1) * (D + 1)))
    mask01 = (np.arange(W)[:, None] <= np.arange(W)[None, :]).astype(F16)
    return {"qTp": qTp, "kT": kTp, "vp": vp, "mask01": mask01}


def _postprocess(results):
    outs = []
    for core in range(NCORES):
        ot = results[core]["outT"].astype(np.float32)  # [4, 65, T]
        o = ot[:, :D, :] / ot[:, D:D + 1, :]           # normalize
        outs.append(o.transpose(0, 2, 1))              # [4, T, 64]
    full = np.concatenate(outs, axis=0)                # [32, T, 64]
    return full.reshape(B, H, T, D).astype(np.float32)


def kernel(q, k, v, _trace=False):
    global last_perf
    from concourse.bass_utils import run_bass_kernel_spmd

    q = np.ascontiguousarray(np.asarray(q), dtype=np.float32)
    k = np.ascontiguousarray(np.asarray(k), dtype=np.float32)
    v = np.ascontiguousarray(np.asarray(v), dtype=np.float32)
    q2 = q.reshape(BH, T, D)
    k2 = k.reshape(BH, T, D)
    v2 = v.reshape(BH, T, D)

    if "nc" not in _nc_cache:
        _nc_cache["nc"] = _build_nc()
    nc = _nc_cache["nc"]

    in_maps = [_prep_core_inputs(q2, k2, v2, core) for core in range(NCORES)]
    res = run_bass_kernel_spmd(
        nc, in_maps, core_ids=list(range(NCORES)), trace=_trace)
    last_perf = res

    return _postprocess(res.results)


# revision 5
# speedup vs baseline: 1.1457x; 1.1457x over previous
"""LocalAttention Bass kernel for Trainium2 (8 NeuronCores) — fp16, software-pipelined (final).

v4 = v3 (fp16 everywhere in SBUF/HBM, fp32 PSUM) + software-pipelined PE
stream + mask on DVE only; PSUM budget 3 score bufs / 2 out bufs and
deeper DMA buffering (q4/k4/v8) — measured ~4-12us better than 2/4 at
every percentile in two interleaved races.

Why: the PE engine queue is in-order. In v3 the per-group program order
was S(g), PV(g), S(g+1), ... so PV(g) — which must wait for exp(g) on
ACT and mask(g) on DVE/GPSIMD — blocked S(g+1) behind it and the PE sat
idle for the whole exp+mask latency every group (~2.5us/group measured,
~160us total). v4 issues S(g+1) BEFORE PV(g) so the PE streams Q@K of
the next group while the softmax of the current one is still on the
other engines. The GPSIMD mask (measured ~2us/group, it is a slow
engine for this) moves to DVE, whose fp16 2x/4x modes make it ~0.2us.

Device algorithm per head and 128-token window w (see v1 docstring):
  S^T[k,q] = K_w' @ [Q_w | Q_{w+1}]   (PSUM pairblock [T1(w) | T0(w+1)])
  P = exp(S^T / sqrt(D)) * causal01   (ACT exp, DVE mask on T1 halves)
  [O^T; r] = [V|1]^T @ P              (2 matmuls/window, ones col = rowsum)
Host divides O^T by r and transposes back.
"""

import numpy as np

F16 = np.float16

B, H, T, D = 4, 8, 8192, 64
W = 128                     # window size
WIN = T // W                # 64 windows per head
NCORES = 8
BH = B * H                  # 32
BH_PER_CORE = BH // NCORES  # 4
NPAIR = BH_PER_CORE // 2    # 2 head pairs per core
CHUNK_W = 32                # windows per load chunk
NCHUNK = WIN // CHUNK_W     # 2
G = 4                       # windows per softmax group (PSUM tile = [128, 1024])
SCALE = float(D) ** -0.5

QBUFS = 4
KBUFS = 4
VBUFS = 8
OBUFS = 3
PBUFS = 8
OPSUM_BUFS = 2
DEPTH = 2   # software-pipeline depth (S of group j+DEPTH issues before PV of j)

_nc_cache = {}
last_perf = None


def _build_nc(skip=(), reps=1):
    import concourse.tile as tile
    from concourse import bacc
    from concourse import mybir
    from contextlib import ExitStack

    f32 = mybir.dt.float32
    f16 = mybir.dt.float16
    Exp = mybir.ActivationFunctionType.Exp
    mult = mybir.AluOpType.mult

    nc = bacc.Bacc()
    qTp = nc.dram_tensor("qTp", [NPAIR, W, (WIN + 1) * W], f16,
                         kind="ExternalInput")
    kT = nc.dram_tensor("kT", [NPAIR, W, T], f16, kind="ExternalInput")
    vp = nc.dram_tensor("vp", [BH_PER_CORE, W, (WIN + 1) * (D + 1)], f16,
                        kind="ExternalInput")
    mask = nc.dram_tensor("mask01", [W, W], f16, kind="ExternalInput")
    outT = nc.dram_tensor("outT", [BH_PER_CORE, D + 1, T], f16,
                          kind="ExternalOutput")

    with tile.TileContext(nc) as tc, ExitStack() as ctx:
        cpool = ctx.enter_context(tc.tile_pool(name="cpool", bufs=1))
        qpool = ctx.enter_context(tc.tile_pool(name="qpool", bufs=QBUFS))
        kpool = ctx.enter_context(tc.tile_pool(name="kpool", bufs=KBUFS))
        vpool = ctx.enter_context(tc.tile_pool(name="vpool", bufs=VBUFS))
        opool = ctx.enter_context(tc.tile_pool(name="opool", bufs=OBUFS))
        ppool = ctx.enter_context(tc.tile_pool(name="ppool", bufs=PBUFS))
        spsum = ctx.enter_context(tc.tile_pool(name="spsum", bufs=3,
                                               space="PSUM"))
        opsum = ctx.enter_context(tc.tile_pool(name="opsum", bufs=OPSUM_BUFS,
                                               space="PSUM"))

        mtile = cpool.tile([W, W], f16)
        nc.sync.dma_start(mtile[:], mask[:])
        z128 = cpool.tile([W, W], f16)       # P for the all-masked pad window
        nc.vector.memset(z128[:], 0.0)

        mm = nc.tensor.matmul

        def emit_front(st, qc, kc, w0, hb):
            """S matmuls + exp + mask for one (group, head) stage."""
            sp = spsum.tile([W, G * 2 * W], f32, tag="sp")
            if "smm" not in skip:
                for i in range(G):
                    wl = w0 + i
                    mm(sp[:, i * 256:(i + 1) * 256],
                       kc[hb:hb + 64, wl * W:(wl + 1) * W],
                       qc[hb:hb + 64, wl * W:(wl + 2) * W],
                       start=True, stop=True)
            pt = ppool.tile([W, G * 2 * W], f16, tag="pt")
            if "exp" not in skip:
                nc.scalar.activation(pt[:], sp[:], Exp, scale=SCALE)
            # causal mask on T1 blocks (cols 0, 256, 512, 768)
            pt3 = pt[:].rearrange("p (g x) -> p g x", x=2 * W)
            t1 = pt3[:, :, 0:W]
            mb = mtile[:, None, :].to_broadcast([W, G, W])
            if "mask" not in skip:
                nc.vector.tensor_tensor(t1, t1, mb, mult)
            st["pt"] = pt

        def emit_back(st, pt_prev, vc, oc, w0, h):
            """PV matmuls + PSUM->SBUF copy for one stage."""
            pt = st["pt"]
            op = opsum.tile([D + 1, G * W], f32, tag="op")
            if "pv" not in skip:
                for i in range(G):
                    wl = w0 + i
                    if i > 0:
                        t0src = pt[:, i * 256 - W:i * 256]
                    elif pt_prev[h] is not None:
                        t0src = pt_prev[h][:, G * 256 - W:G * 256]
                    else:
                        t0src = z128[:]
                    mm(op[:, i * W:(i + 1) * W],
                       vc[:, wl * (D + 1):(wl + 1) * (D + 1)],
                       t0src, start=True, stop=False)
                    mm(op[:, i * W:(i + 1) * W],
                       vc[:, (wl + 1) * (D + 1):(wl + 2) * (D + 1)],
                       pt[:, i * 256:i * 256 + W],
                       start=False, stop=True)
            if "ocopy" not in skip:
                nc.vector.tensor_copy(oc[:, w0 * W:(w0 + G) * W], op[:])
            pt_prev[h] = pt

        def body():
            for p in range(NPAIR):
                pt_prev = [None, None]
                for c in range(NCHUNK):
                    c0 = c * CHUNK_W * W
                    qc = qpool.tile([W, (CHUNK_W + 1) * W], f16, tag="qc")
                    kc = kpool.tile([W, CHUNK_W * W], f16, tag="kc")
                    if "loads" not in skip:
                        nc.sync.dma_start(
                            qc[:], qTp[p, :, c0:c0 + (CHUNK_W + 1) * W])
                        nc.sync.dma_start(kc[:], kT[p, :, c0:c0 + CHUNK_W * W])
                    vcs = []
                    ocs = []
                    for h in range(2):
                        vc = vpool.tile([W, (CHUNK_W + 1) * (D + 1)], f16,
                                        tag="vc")
                        if "loads" not in skip:
                            v0 = c * CHUNK_W * (D + 1)
                            nc.sync.dma_start(
                                vc[:],
                                vp[2 * p + h, :,
                                   v0:v0 + (CHUNK_W + 1) * (D + 1)])
                        vcs.append(vc)
                        oc = opool.tile([D + 1, CHUNK_W * W], f16, tag="oc")
                        ocs.append(oc)

                    # software pipeline: front(j+1) issues before back(j) so
                    # the PE streams next group's S while this group's
                    # exp/mask are still on ACT/DVE.
                    stages = [(g, h) for g in range(CHUNK_W // G)
                              for h in range(2)]
                    st = [{} for _ in stages]
                    d = DEPTH
                    for idx in range(len(stages) + d):
                        if idx < len(stages):
                            g, h = stages[idx]
                            emit_front(st[idx], qc, kc, g * G, h * 64)
                        if idx >= d:
                            g, h = stages[idx - d]
                            emit_back(st[idx - d], pt_prev, vcs[h], ocs[h],
                                      g * G, h)

                    if "store" not in skip:
                        for h in range(2):
                            nc.sync.dma_start(
                                outT[2 * p + h, :, c0:c0 + CHUNK_W * W],
                                ocs[h][:])

        if reps > 1:
            engs = (mybir.EngineType.PE, mybir.EngineType.Activation,
                    mybir.EngineType.DVE, mybir.EngineType.SP,
                    mybir.EngineType.Pool)
            with tc.For_i(0, reps, 1, hint_engines=engs):
                body()
        else:
            body()
    nc.finalize()
    return nc


def _prep_core_inputs(q2, k2, v2, core):
    s0 = core * BH_PER_CORE
    qTp = np.zeros((NPAIR, W, (WIN + 1) * W), F16)
    kTp = np.zeros((NPAIR, W, T), F16)
    for p in range(NPAIR):
        for h in range(2):
            bh = s0 + 2 * p + h
            qTp[p, h * 64:(h + 1) * 64, :T] = q2[bh].T.astype(F16)
            kTp[p, h * 64:(h + 1) * 64, :] = k2[bh].T.astype(F16)
    vr = v2[s0:s0 + BH_PER_CORE].reshape(
        BH_PER_CORE, WIN, W, D).transpose(0, 2, 1, 3)
    vp = np.zeros((BH_PER_CORE, W, WIN + 1, D + 1), F16)
    vp[:, :, 1:, :D] = vr.astype(F16)
    vp[:, :, :, D] = 1.0
    vp = np.ascontiguousarray(vp.reshape(BH_PER_CORE, W, (WIN + 1) * (D + 1)))
    mask01 = (np.arange(W)[:, None] <= np.arange(W)[None, :]).astype(F16)
    return {"qTp": qTp, "kT": kTp, "vp": vp, "mask01": mask01}


def _postprocess(results):
    outs = []
    for core in range(NCORES):
        ot = results[core]["outT"].astype(np.float32)  # [4, 65, T]
        o = ot[:, :D, :] / ot[:, D:D + 1, :]           # normalize
        outs.append(o.transpose(0, 2, 1))              # [4, T, 64]
    full = np.concatenate(outs, axis=0)                # [32, T, 64]
    return full.reshape(B, H, T, D).astype(np.float32)


def kernel(q, k, v, _trace=False):
    global last_perf
    from concourse.bass_utils import run_bass_kernel_spmd

    q = np.ascontiguousarray(np.asarray(q), dtype=np.float32)
    k = np.ascontiguousarray(np.asarray(k), dtype=np.float32)
    v = np.ascontiguousarray(np.asarray(v), dtype=np.float32)
    q2 = q.reshape(BH, T, D)
    k2 = k.reshape(BH, T, D)
    v2 = v.reshape(BH, T, D)

    if "nc" not in _nc_cache:
        _nc_cache["nc"] = _build_nc()
    nc = _nc_cache["nc"]

    in_maps = [_prep_core_inputs(q2, k2, v2, core) for core in range(NCORES)]
    res = run_bass_kernel_spmd(
        nc, in_maps, core_ids=list(range(NCORES)), trace=_trace)
    last_perf = res

    return _postprocess(res.results)
